# revision 3
# baseline (speedup 1.0000x reference)
"""Trainium2 Bass kernel for GQA multi-head attention (RoPE + padding|causal mask).

Sequence-sharded, collective-free design: 8 cores = 2 (batch) x 4 (query
windows of 512 rows). Each core computes K/V for the full sequence (all 8
KV heads), Q for its own 512-row window (all 16 q heads), attention for all
heads over its window, and its own [512, 1024] slice of the output
projection. No inter-core communication; the host only slices inputs and
concatenates the 8 output row-blocks.

All per-core differences are data, not program: xq (the core's x columns),
cosQ/sinQ (window RoPE tables), and maskM (per-core visibility mask
multiplied into the exp'd scores). One SPMD program serves all 8 cores.

Layouts (transposed, no on-device transposes):
  kT[4 tiles][128, 2048]: tile t rows = kv head t (deinterleaved RoPE pairs,
    32-row blocks) | kv head t+4. qT[8 tiles][128, 512]: tile p = q head p |
    q head p+8 (so score matmuls pack 2 heads into PE row groups 0-63/64-127
    with K=64 each). vv[128, 16*520]: natural-layout V per sk tile, 8 heads
    x (64 cols + ones col) -- the ones column makes the softmax denominator
    ride row 64 of the AV matmul for free. exp on ScalarE reads PSUM with
    the 1/sqrt(dk) scale fused; mask is one bf16 multiply on the exp'd
    scores (uniform across tiles -> uniform program).
"""

import sys

if "/opt/trn_rl_repo" not in sys.path:
    sys.path.insert(0, "/opt/trn_rl_repo")

import numpy as np
import ml_dtypes

BF_NP = ml_dtypes.bfloat16

import concourse.mybir as mybir
import concourse.tile as tile
from concourse import bacc
from concourse.bass_utils import run_bass_kernel_spmd

B, S, D = 2, 2048, 1024
H_Q, H_KV, DK, DV = 16, 8, 64, 64
N_CORES = 8
P = 128
W = 512          # query window per core
FP = mybir.dt.float32
BF = mybir.dt.bfloat16
F8 = mybir.dt.float8e4
F8_NP = ml_dtypes.float8_e4m3
SCALE = 1.0 / 8.0  # 1/sqrt(DK)
NK = D // P      # 8 k-tiles over the model dim
NSK = S // P     # 16 sk tiles
VC = DV + 1      # 65 = V cols + ones col per head
VROW = H_KV * VC  # 520 cols of vv per sk tile


def build_nc():
    nc = bacc.Bacc("TRN2", target_bir_lowering=False, debug=False,
                   num_devices=1)

    xT = nc.dram_tensor("xT", [D, S], BF, kind="ExternalInput")
    xq = nc.dram_tensor("xq", [D, W], BF, kind="ExternalInput")
    wq = nc.dram_tensor("wq", [D, H_Q * DK], BF, kind="ExternalInput")
    wk = nc.dram_tensor("wk", [D, H_KV * DK], BF, kind="ExternalInput")
    wv = nc.dram_tensor("wv", [D, VROW], BF, kind="ExternalInput")
    ev = nc.dram_tensor("ev", [1, VROW], BF, kind="ExternalInput")
    wo = nc.dram_tensor("wo", [H_Q * DV, D], BF, kind="ExternalInput")
    bo = nc.dram_tensor("bo", [1, D], BF, kind="ExternalInput")
    bq2 = nc.dram_tensor("bq2", [1, H_Q * DK], BF, kind="ExternalInput")
    bk1 = nc.dram_tensor("bk1", [1, H_KV * DK], BF, kind="ExternalInput")
    cosA = nc.dram_tensor("cosA", [P, S], BF, kind="ExternalInput")
    sinA = nc.dram_tensor("sinA", [P, S], BF, kind="ExternalInput")
    cosQ = nc.dram_tensor("cosQ", [P, W], BF, kind="ExternalInput")
    sinQ = nc.dram_tensor("sinQ", [P, W], BF, kind="ExternalInput")
    maskM = nc.dram_tensor("maskM", [P, NSK * W], BF, kind="ExternalInput")
    ones = nc.dram_tensor("ones", [1, W], BF, kind="ExternalInput")
    outp = nc.dram_tensor("outp", [W, D], FP, kind="ExternalOutput")

    Exp = mybir.ActivationFunctionType.Exp

    with tile.TileContext(nc) as tc:
        with (
            tc.tile_pool(name="persist", bufs=1) as pp,
            tc.tile_pool(name="psum", bufs=1, space="PSUM") as ps,
            tc.tile_pool(name="p1", bufs=1) as p1,
            tc.tile_pool(name="rope_tmp", bufs=1) as rt,
            tc.tile_pool(name="exp_pool", bufs=4) as epool,
            tc.tile_pool(name="norm_pool", bufs=1) as npo,
            tc.tile_pool(name="out_pool", bufs=2) as op,
        ):
            # ---- persistent tiles ----
            xts = [pp.tile([P, S], BF, tag=f"xT{k}", name=f"xT{k}")
                   for k in range(NK)]
            kts = [pp.tile([P, S], BF, tag=f"kT{t}", name=f"kT{t}")
                   for t in range(4)]
            qts = [pp.tile([P, W], BF, tag=f"qT{p}", name=f"qT{p}")
                   for p in range(8)]
            vv = pp.tile([P, NSK * VROW], BF, tag="vv")
            aos = [pp.tile([P, W], BF, tag=f"ao{p}", name=f"ao{p}")
                   for p in range(8)]
            msk = pp.tile([P, NSK * W], BF, tag="msk")
            ones_sb = pp.tile([1, W], BF, tag="ones")
            bo_sb = pp.tile([1, D], BF, tag="bo")

            # ---- phase-1 tiles ----
            wqs = [p1.tile([P, H_Q * DK], BF, tag=f"wq{k}", name=f"wq{k}")
                   for k in range(NK)]
            wks = [p1.tile([P, H_KV * DK], BF, tag=f"wk{k}", name=f"wk{k}")
                   for k in range(NK)]
            wvs = [p1.tile([P, VROW], BF, tag=f"wv{k}", name=f"wv{k}")
                   for k in range(NK)]
            xqs = [p1.tile([P, W], BF, tag=f"xq{k}", name=f"xq{k}")
                   for k in range(NK)]
            cos_sb = p1.tile([P, S], BF, tag="cos")
            sin_sb = p1.tile([P, S], BF, tag="sin")
            cq_sb = p1.tile([P, W], BF, tag="cq")
            sq_sb = p1.tile([P, W], BF, tag="sq")
            ev_sb = p1.tile([1, VROW], BF, tag="ev")
            bq_sb = p1.tile([1, H_Q * DK], BF, tag="bq")
            bk_sb = p1.tile([1, H_KV * DK], BF, tag="bk")

            # ---- input DMAs, prefetch-ordered by first use, two queues ----
            nc.sync.dma_start(ones_sb[:], ones[:])
            nc.sync.dma_start(bk_sb[:], bk1[:])
            nc.sync.dma_start(bq_sb[:], bq2[:])
            nc.sync.dma_start(cq_sb[:], cosQ[:])
            nc.sync.dma_start(sq_sb[:], sinQ[:])
            for k in range(4, NK):
                nc.scalar.dma_start(xts[k][:], xT[k * P:(k + 1) * P, :])
            nc.scalar.dma_start(cos_sb[:], cosA[:])
            nc.scalar.dma_start(sin_sb[:], sinA[:])
            for k in range(NK):
                nc.sync.dma_start(wks[k][:], wk[k * P:(k + 1) * P, :])
                if k < 4:
                    nc.sync.dma_start(xts[k][:], xT[k * P:(k + 1) * P, :])
            for k in range(NK):
                # q columns for pairs 0/1 first; the rest on the slow queue
                nc.sync.dma_start(wqs[k][:, 0:256],
                                  wq[k * P:(k + 1) * P, 0:256])
                nc.sync.dma_start(xqs[k][:], xq[k * P:(k + 1) * P, :])
            nc.sync.dma_start(ev_sb[:], ev[:])
            nc.sync.dma_start(msk[:, 0:4 * W], maskM[:, 0:4 * W])
            for k in range(NK):
                nc.scalar.dma_start(wvs[k][:], wv[k * P:(k + 1) * P, :])
            nc.scalar.dma_start(msk[:, 4 * W:NSK * W], maskM[:, 4 * W:NSK * W])
            for k in range(NK):
                nc.scalar.dma_start(wqs[k][:, 256:1024],
                                    wq[k * P:(k + 1) * P, 256:1024])
            nc.scalar.dma_start(bo_sb[:], bo[:])

            def rope_block(srcs, c_ap, s_ap, dst, ncols):
                """RoPE a [128, ncols] projection (psum sources given as a
                list of (ap, col_offset, width) covering ncols). Rows are 4
                32-row blocks (x1,x2,x1,x2); rotation partner is the 32-row
                neighbour, fetched via an SBUF->SBUF DMA swap. The sin table
                is pre-negated on x2 rows (host side) so the combine is a
                single full-width add: out = x*cos + swap(x*sin')."""
                t_sb = rt.tile([P, 1024], BF, tag="ropeT", name="ropeT",
                               bufs=2)
                s_sb = rt.tile([P, 1024], BF, tag="ropeS", name="ropeS",
                             bufs=2)
                ss = rt.tile([P, 1024], BF, tag="ropeSS", name="ropeSS",
                             bufs=2)
                for ap, co, cw in srcs:
                    nc.vector.tensor_mul(t_sb[:, co:co + cw], ap,
                                         c_ap[:, co:co + cw])
                    nc.vector.tensor_mul(s_sb[:, co:co + cw], ap,
                                         s_ap[:, co:co + cw])
                for blk in range(4):
                    r0, rs = blk * 32, (blk ^ 1) * 32
                    nc.sync.dma_start(ss[r0:r0 + 32, 0:ncols],
                                      s_sb[rs:rs + 32, 0:ncols])
                nc.vector.tensor_add(dst, t_sb[:, 0:ncols], ss[:, 0:ncols])

            def emit_k(t):
                """K projection + RoPE for kT tile t = [kv_t | kv_{t+4}]."""
                for half in range(2):     # 1024 seq cols per rope call
                    ho = half * 1024
                    srcs = []
                    for n in range(2):
                        pk = ps.tile([P, 512], FP, tag="B", name="pk",
                                     bufs=2, padded_shape=[P, 512])
                        nc.tensor.matmul(pk[:],
                                         bk_sb[:, t * P:(t + 1) * P],
                                         ones_sb[:], start=True, stop=False)
                        for k in range(NK):
                            nc.tensor.matmul(
                                pk[:], wks[k][:, t * P:(t + 1) * P],
                                xts[k][:, ho + n * 512:ho + (n + 1) * 512],
                                start=False, stop=(k == NK - 1))
                        srcs.append((pk[:], n * 512, 512))
                    rope_block(srcs, cos_sb[:, ho:ho + 1024],
                               sin_sb[:, ho:ho + 1024],
                               kts[t][:, ho:ho + 1024], 1024)

            def emit_q1(p):
                """Q projection + RoPE for qT tile p."""
                pq = ps.tile([P, 512], FP, tag="B", name="pq", bufs=2,
                             padded_shape=[P, 512])
                nc.tensor.matmul(pq[:], bq_sb[:, p * P:(p + 1) * P],
                                 ones_sb[:], start=True, stop=False)
                for k in range(NK):
                    nc.tensor.matmul(pq[:],
                                     wqs[k][:, p * P:(p + 1) * P],
                                     xqs[k][:], start=False,
                                     stop=(k == NK - 1))
                rope_block([(pq[:], 0, 512)], cq_sb[:], sq_sb[:],
                           qts[p][:], 512)

            def emit_v(i):
                """V projection for sk tile i (natural layout)."""
                for hh in range(2):       # 260 cols per half (4 heads)
                    col = slice(hh * 260, (hh + 1) * 260)
                    pv = ps.tile([P, 260], FP, tag="B", name="pv", bufs=2,
                                 padded_shape=[P, 512])
                    nc.tensor.matmul(pv[:], ones_sb[:, 0:P], ev_sb[:, col],
                                     start=True, stop=False)
                    for k in range(NK):
                        nc.tensor.matmul(pv[:],
                                         xts[k][:, i * P:(i + 1) * P],
                                         wvs[k][:, col], start=False,
                                         stop=(k == NK - 1))
                    nc.vector.tensor_copy(
                        vv[:, i * VROW + hh * 260:i * VROW + (hh + 1) * 260],
                        pv[:])

            def emit_pair(p, with_v=False, steps=None):
                """Attention for q heads (p, p+8). steps: {i: [callables]}
                emitted at the top of iteration i to interleave other work
                into the in-order engine streams."""
                t = p // 2                # kT tile: kv p//2 | kv p//2+4
                av = [ps.tile([VC, 512], FP, tag="C", bufs=2,
                              padded_shape=[P, 512], name=f"av{h}")
                      for h in range(2)]
                for i in range(NSK):
                    for fn in (steps or {}).get(i, ()):
                        fn()
                    if with_v:
                        emit_v(i)
                    sc = ps.tile([P, 1024], FP, tag="A", name="sc", bufs=2)
                    for h in range(2):
                        r0 = h * 64
                        nc.tensor.matmul(
                            sc[:, h * 512:(h + 1) * 512],
                            kts[t][r0:r0 + 64, i * P:(i + 1) * P],
                            qts[p][r0:r0 + 64, :],
                            start=True, stop=True)
                    e = epool.tile([P, 1024], BF, tag="e", name="e", bufs=4)
                    nc.scalar.activation(e[:], sc[:], Exp, scale=SCALE)
                    e3 = e[:].rearrange("r (h w) -> r h w", h=2)
                    m3 = msk[:, i * W:(i + 1) * W].unsqueeze(1).broadcast_to(
                        [P, 2, W])
                    nc.vector.tensor_mul(e3, e3, m3)
                    for h in range(2):
                        kv = t + h * 4    # kv head for q head p + h*8
                        vsl = slice(i * VROW + kv * VC,
                                    i * VROW + kv * VC + VC)
                        nc.tensor.matmul(av[h][:], vv[:, vsl],
                                         e[:, h * 512:(h + 1) * 512],
                                         start=(i == 0), stop=(i == NSK - 1),
                                         skip_group_check=True)
                # normalize: denominator rides row 64 of av
                avs = npo.tile([VC, 1024], FP, tag="avs", name="avs")
                rc = npo.tile([1, 1024], FP, tag="rc", name="rc")
                bcs = npo.tile([64, 1024], FP, tag="bcs", name="bcs")
                st1 = npo.tile([64, 512], BF, tag="st1", name="st1")
                for h in range(2):
                    nc.vector.tensor_copy(avs[:, h * 512:(h + 1) * 512],
                                          av[h][:])
                nc.sync.dma_start(rc[0:1, :], avs[64:65, :])
                nc.gpsimd.partition_broadcast(bcs[:], rc[0:1, :])
                nc.vector.reciprocal(bcs[:], bcs[:])
                nc.vector.tensor_mul(aos[p][0:64, :], avs[0:64, 0:512],
                                     bcs[:, 0:512])
                nc.vector.tensor_mul(st1[:], avs[0:64, 512:1024],
                                     bcs[:, 512:1024])
                nc.sync.dma_start(aos[p][64:128, :], st1[:])

            # ---- output projection helpers ----
            wo_t = []

            def po_partial(m, nh):
                po = ps.tile([P, 512], FP, tag="B", name="po", bufs=2,
                             padded_shape=[P, 512])
                nsl = slice(nh * 512, (nh + 1) * 512)
                nc.tensor.matmul(po[:], ones_sb[:, 0:P], bo_sb[:, nsl],
                                 start=True, stop=False)
                for k in range(NK - 1):
                    nc.tensor.matmul(
                        po[:], aos[k][:, m * P:(m + 1) * P],
                        wo_t[k][:, nsl], start=False, stop=False)
                return po

            def po_finish(m, nh, po):
                nsl = slice(nh * 512, (nh + 1) * 512)
                nc.tensor.matmul(
                    po[:], aos[NK - 1][:, m * P:(m + 1) * P],
                    wo_t[NK - 1][:, nsl], start=False, stop=True)
                osb = op.tile([P, 512], FP, tag="osb", name="osb")
                nc.scalar.copy(osb[:], po[:])
                nc.sync.dma_start(outp[m * P:(m + 1) * P, nsl], osb[:])

            # ---- interleaved emission: ACT streams exps early, K spread ----
            emit_k(0)
            emit_q1(0)
            emit_q1(1)
            emit_pair(0, with_v=True,
                      steps={2: [lambda: emit_q1(2)], 4: [lambda: emit_q1(3)],
                             6: [lambda: emit_q1(4)], 8: [lambda: emit_q1(5)],
                             10: [lambda: emit_q1(6)],
                             12: [lambda: emit_q1(7)]})
            # wo reuses the wq slots (same tag+shape, emitted after the last
            # Q-projection read so the WAR dependency orders correctly).
            wo_t.extend(p1.tile([P, D], BF, tag=f"wq{k}", name=f"wo{k}")
                        for k in range(NK))
            for k in range(NK):
                nc.scalar.dma_start(wo_t[k][:], wo[k * P:(k + 1) * P, :])
            emit_pair(1)
            emit_k(1)
            emit_pair(2)
            emit_pair(3)
            emit_k(2)
            emit_pair(4)
            emit_pair(5)
            emit_k(3)
            emit_pair(6)
            pos = {}
            emit_pair(7, steps={
                4: [lambda: pos.setdefault(0, po_partial(0, 0))],
                10: [lambda: pos.setdefault(1, po_partial(0, 1))]})
            po_finish(0, 0, pos[0])
            po_finish(0, 1, pos[1])
            for m in range(1, 4):
                for nh in range(2):
                    po = po_partial(m, nh)
                    po_finish(m, nh, po)

    nc.compile()
    return nc


def _deint_cols(Wm, h, dh):
    cols = Wm[:, h * dh:(h + 1) * dh]
    return np.concatenate([cols[:, 0::2], cols[:, 1::2]], axis=1)


def _deint_vec(v, h, dh):
    seg = v[h * dh:(h + 1) * dh]
    return np.concatenate([seg[0::2], seg[1::2]])


def _prep_shared(Wq, bq, Wk, bk, Wv, bv, Wo, bo, freqs_cos, freqs_sin):
    """Host-side weight permutations shared by all cores."""
    f32 = np.float32
    bf = lambda a: np.ascontiguousarray(np.asarray(a, BF_NP))
    # q head order per m-tile p: [p | p+8]
    qorder = [h for p in range(8) for h in (p, p + 8)]
    # kv head order per kT tile t: [t | t+4]
    korder = [h for t in range(4) for h in (t, t + 4)]
    Wq_, Wk_, Wv_ = (np.asarray(Wq, f32), np.asarray(Wk, f32),
                     np.asarray(Wv, f32))
    wq_p = np.concatenate([_deint_cols(Wq_, h, DK) for h in qorder], axis=1)
    wk_p = np.concatenate([_deint_cols(Wk_, h, DK) for h in korder], axis=1)
    bq_p = np.concatenate([_deint_vec(np.asarray(bq, f32), h, DK)
                           for h in qorder]).reshape(1, H_Q * DK)
    bk_p = np.concatenate([_deint_vec(np.asarray(bk, f32), h, DK)
                           for h in korder]).reshape(1, H_KV * DK)
    zero = np.zeros((D, 1), f32)
    wv_p = np.concatenate(
        [arr for h in range(H_KV)
         for arr in (Wv_[:, h * DV:(h + 1) * DV], zero)], axis=1)
    bv_ = np.asarray(bv, f32)
    ev = np.concatenate(
        [arr for h in range(H_KV)
         for arr in (bv_[h * DV:(h + 1) * DV], [1.0])]).astype(f32)
    # Wo rows reordered to match aoT row order (q head p rows 0-63 of tile p,
    # q head p+8 rows 64-127)
    Wo_ = np.asarray(Wo, f32).reshape(H_Q, DV, D)
    wo_p = Wo_[qorder].reshape(H_Q * DV, D)
    cosA = np.tile(np.asarray(freqs_cos, f32).T, (4, 1))  # [128, S]
    sinA = np.tile(np.asarray(freqs_sin, f32).T, (4, 1)).copy()
    # pre-negate sin on x2 rows so RoPE's combine is a single add
    sinA[32:64] *= -1.0
    sinA[96:128] *= -1.0
    return {
        "wq": bf(wq_p), "wk": bf(wk_p), "wv": bf(wv_p),
        "ev": bf(ev.reshape(1, VROW)), "wo": bf(wo_p),
        "bo": bf(np.asarray(bo, f32).reshape(1, D)),
        "bq2": bf(bq_p), "bk1": bf(bk_p),
        "cosA": bf(cosA),
        "sinA": bf(sinA),
        "ones": bf(np.ones((1, W), f32)),
    }


def _prep_core_inputs(shared, xT_b, pad_b, cosA, sinA, j):
    """Per-core inputs for query window [j*512, (j+1)*512)."""
    f32 = np.float32
    o = j * W
    xq = np.ascontiguousarray(xT_b[:, o:o + W])
    # visibility: pad[k] OR k <= q  (k = i*128 + r, q = o + c)
    r = np.arange(P)[:, None, None]
    i = np.arange(NSK)[None, :, None]
    c = np.arange(W)[None, None, :]
    kidx = i * P + r
    vis = pad_b[kidx.reshape(P, -1)].reshape(P, NSK, 1) | (kidx <= o + c)
    maskM = np.ascontiguousarray(
        vis.astype(f32).reshape(P, NSK * W).astype(BF_NP))
    d = dict(shared)
    d.update({
        "xT": xT_b, "xq": xq,
        "cosQ": np.ascontiguousarray(cosA[:, o:o + W]),
        "sinQ": np.ascontiguousarray(sinA[:, o:o + W]),
        "maskM": maskM,
    })
    return d


_NC_CACHE = {}


def _get_nc():
    if "nc" not in _NC_CACHE:
        _NC_CACHE["nc"] = build_nc()
    return _NC_CACHE["nc"]


def _make_in_maps(x, Wq, bq, Wk, bk, Wv, bv, Wo, bo, freqs_cos, freqs_sin,
                  attention_mask):
    shared = _prep_shared(Wq, bq, Wk, bk, Wv, bv, Wo, bo, freqs_cos,
                          freqs_sin)
    xTs = [np.ascontiguousarray(
        np.asarray(x[b], np.float32).T.astype(BF_NP)) for b in range(B)]
    pads = [np.asarray(attention_mask[b]).astype(bool) for b in range(B)]
    in_maps = []
    for cix in range(N_CORES):
        b, j = cix // 4, cix % 4
        in_maps.append(_prep_core_inputs(shared, xTs[b], pads[b],
                                         shared["cosA"], shared["sinA"], j))
    return in_maps


def kernel(x, Wq, bq, Wk, bk, Wv, bv, Wo, bo, freqs_cos, freqs_sin,
           attention_mask):
    nc = _get_nc()
    in_maps = _make_in_maps(x, Wq, bq, Wk, bk, Wv, bv, Wo, bo, freqs_cos,
                            freqs_sin, attention_mask)
    res = run_bass_kernel_spmd(nc, in_maps, core_ids=list(range(N_CORES)))
    out = np.empty((B, S, D), np.float32)
    for cix in range(N_CORES):
        b, j = cix // 4, cix % 4
        out[b, j * W:(j + 1) * W, :] = res.results[cix]["outp"]
    return out


if __name__ == "__main__":
    rng = np.random.default_rng(0)
    ins = {
        "x": rng.standard_normal((B, S, D), dtype=np.float32),
        "Wq": rng.standard_normal((D, H_Q * DK), dtype=np.float32) * 0.02,
        "bq": np.zeros(H_Q * DK, np.float32),
        "Wk": rng.standard_normal((D, H_KV * DK), dtype=np.float32) * 0.02,
        "bk": np.zeros(H_KV * DK, np.float32),
        "Wv": rng.standard_normal((D, H_KV * DV), dtype=np.float32) * 0.02,
        "bv": np.zeros(H_KV * DV, np.float32),
        "Wo": rng.standard_normal((H_Q * DV, D), dtype=np.float32) * 0.02,
        "bo": np.zeros(D, np.float32),
        "freqs_cos": rng.standard_normal((S, DK // 2), dtype=np.float32),
        "freqs_sin": rng.standard_normal((S, DK // 2), dtype=np.float32),
        "attention_mask": rng.random((B, S)) < 0.9,
    }
    out = kernel(**ins)
    print("ran, out shape", out.shape, "finite:", np.isfinite(out).all())
